# revision 1
# baseline (speedup 1.0000x reference)
"""Gaussian kernel matrix K = exp(-|xi-xj|^2/2) on 8 TRN2 NeuronCores.

Input : points [4, 4096, 64] f32
Output: K      [4, 4096, 4096] f32

Sharding: core c handles batch c//2, row half c%2 (2048 rows x 4096 cols,
32 MB f32 out per core).

Math: -d2/2 = xi.xj - |xi|^2/2 - |xj|^2/2.
The matmul computes  inner_aug[i,j] = xi.xj - |xj|^2/2  via an augmented
contraction row (lhsT row 64 = ones, rhs row 64 = -|xj|^2/2, K=65).
The -|xi|^2/2 term is applied as the per-partition fp32 bias of the
ScalarE activation:  K = Exp(1.0*psum + bias_i).

Matmul precision: points are split into bf16 hi + lo (x = hi + lo), and
three bf16 matmuls accumulate hi*hi + hi*lo + lo*hi in fp32 PSUM (the
dropped lo*lo term is ~1e-5 relative). This runs at bf16 PE speed instead
of fp32's 4-cycles-per-row.
"""

import numpy as np
import ml_dtypes

B, N, D = 4, 4096, 64
KA = D + 1          # augmented contraction dim (ones / -sq/2 row)
HALF = N // 2       # rows per core
N_CORES = 8
IT = HALF // 128    # 16 i-tiles of 128 partitions
JG = N // 2048      # 2 j-groups of 2048 (4 PSUM banks)
JS = 2048 // 512    # 4 matmul sub-tiles per j-group

_cache = {}


NBUF_OUT = 3   # SBUF staging buffers for the output tiles
NGRP = IT * JG  # 32 pipeline groups of [128, 2048]


def _build_nc():
    import concourse.bass as bass
    import concourse.mybir as mybir

    f32 = mybir.dt.float32
    bf16 = mybir.dt.bfloat16
    Exp = mybir.ActivationFunctionType.Exp

    nc = bass.Bass()
    xl_hi_d = nc.dram_tensor("xl_hi", [KA, HALF], bf16, kind="ExternalInput")
    xl_lo_d = nc.dram_tensor("xl_lo", [KA, HALF], bf16, kind="ExternalInput")
    xr_hi_d = nc.dram_tensor("xr_hi", [KA, N], bf16, kind="ExternalInput")
    xr_lo_d = nc.dram_tensor("xr_lo", [KA, N], bf16, kind="ExternalInput")
    bias_d = nc.dram_tensor("bias", [128, IT], f32, kind="ExternalInput")
    out_d = nc.dram_tensor("out", [HALF, N], f32, kind="ExternalOutput")

    with (
        nc.sbuf_tensor([KA, HALF], bf16) as xl_hi,
        nc.sbuf_tensor([KA, HALF], bf16) as xl_lo,
        nc.sbuf_tensor([KA, N], bf16) as xr_hi,
        nc.sbuf_tensor([KA, N], bf16) as xr_lo,
        nc.sbuf_tensor([128, IT], f32) as bias,
        nc.sbuf_tensor([128, NBUF_OUT * 4096], f32) as ot_buf,
        nc.psum_tensor([128, 2048], f32) as ps0,
        nc.psum_tensor([128, 2048], f32) as ps1,
        nc.semaphore("in_sem") as in_sem,
        nc.semaphore("mm_sem") as mm_sem,
        nc.semaphore("act_sem") as act_sem,
        nc.semaphore("out_sem_a") as out_sem_a,
        nc.semaphore("out_sem_b") as out_sem_b,
        nc.Block() as block,
    ):
        pss = [ps0, ps1]

        def ot(g):
            # out staging: one [128, 4096] buffer per it-block, slot it%NBUF
            it, jg = divmod(g, JG)
            s = (it % NBUF_OUT) * 4096 + jg * 2048
            return ot_buf[:, s : s + 2048]

        @block.sync
        def _(sync):
            sync.dma_start(out=xl_hi[:], in_=xl_hi_d[:, :]).then_inc(in_sem, 16)
            sync.dma_start(out=xl_lo[:], in_=xl_lo_d[:, :]).then_inc(in_sem, 16)
            sync.dma_start(out=xr_hi[:], in_=xr_hi_d[:, :]).then_inc(in_sem, 16)
            sync.dma_start(out=xr_lo[:], in_=xr_lo_d[:, :]).then_inc(in_sem, 16)
            sync.dma_start(out=bias[:], in_=bias_d[:, :]).then_inc(in_sem, 16)
            for it in range(IT):
                # jg=0 half of each it-block goes out on the SP HWDGE ring;
                # the jg=1 half is issued from the ACT engine's ring below.
                sync.wait_ge(act_sem, JS * JG * it + JS)
                s = (it % NBUF_OUT) * 4096
                sync.dma_start(
                    out=out_d[it * 128 : (it + 1) * 128, 0:2048],
                    in_=ot_buf[:, s : s + 2048],
                ).then_inc(out_sem_a, 16)

        @block.tensor
        def _(tensor):
            tensor.wait_ge(in_sem, 80)
            for g in range(NGRP):
                it, jg = divmod(g, JG)
                if g >= 2:
                    # psum slot g%2 was last read by group g-2's activations
                    tensor.wait_ge(act_sem, JS * (g - 1))
                ps = pss[g % 2]
                lh = xl_hi[:, it * 128 : (it + 1) * 128]
                ll = xl_lo[:, it * 128 : (it + 1) * 128]
                last = None
                # Pass-major order: the stationary operand changes only twice
                # per group (lh -> ll), so the rust layer dedupes LDWEIGHTS.
                passes = [(lh, xr_hi, True, False),
                          (lh, xr_lo, False, False),
                          (ll, xr_hi, False, True)]
                for w, rsrc, st, sp in passes:
                    for js in range(JS):
                        c0 = jg * 2048 + js * 512
                        dst = ps[:, js * 512 : (js + 1) * 512]
                        last = tensor.matmul(dst, w, rsrc[:, c0 : c0 + 512],
                                             start=st, stop=sp)
                # PE completes in order; one inc on the last matmul
                last.then_inc(mm_sem, 1)

        @block.scalar
        def _(scalar):
            scalar.wait_ge(in_sem, 80)
            for g in range(NGRP):
                it, jg = divmod(g, JG)
                ps = pss[g % 2]
                scalar.wait_ge(mm_sem, g + 1)
                if jg == 0 and it >= NBUF_OUT:
                    # out slot it%NBUF_OUT last read by DMAs of it-NBUF_OUT
                    scalar.wait_ge(out_sem_a, 16 * (it - NBUF_OUT + 1))
                    scalar.wait_ge(out_sem_b, 16 * (it - NBUF_OUT + 1))
                o = ot(g)
                for js in range(JS):
                    sl = slice(js * 512, (js + 1) * 512)
                    scalar.activation(
                        o[:, sl], ps[:, sl], Exp,
                        bias=bias[:, it : it + 1], scale=1.0,
                    ).then_inc(act_sem, 1)
                if jg == 1:
                    # own-engine wait: ensure this block's exps completed
                    # before the ACT-ring DMA reads the staging buffer
                    scalar.wait_ge(act_sem, JS * JG * (it + 1))
                    s = (it % NBUF_OUT) * 4096
                    scalar.dma_start(
                        out=out_d[it * 128 : (it + 1) * 128, 2048:4096],
                        in_=ot_buf[:, s + 2048 : s + 4096],
                    ).then_inc(out_sem_b, 16)
    return nc


def _get_nc():
    if "nc" not in _cache:
        _cache["nc"] = _build_nc()
    return _cache["nc"]


def _prep_inputs(points: np.ndarray):
    """Host-side shard/layout prep: per-core transposed + augmented operands."""
    bf16 = ml_dtypes.bfloat16
    points = np.asarray(points, dtype=np.float32)
    in_maps = []
    for c in range(N_CORES):
        b, h = divmod(c, 2)
        x = points[b]                              # [N, D]
        sq = np.sum(x * x, axis=1)                 # [N]
        xt = np.ascontiguousarray(x.T)             # [D, N]

        xr = np.empty((KA, N), np.float32)
        xr[:D] = xt
        xr[D] = -0.5 * sq

        rows = slice(h * HALF, (h + 1) * HALF)
        xl = np.empty((KA, HALF), np.float32)
        xl[:D] = xt[:, rows]
        xl[D] = 1.0

        xr_hi = xr.astype(bf16)
        xr_lo = (xr - xr_hi.astype(np.float32)).astype(bf16)
        xl_hi = xl.astype(bf16)
        xl_lo = (xl - xl_hi.astype(np.float32)).astype(bf16)

        bias = np.ascontiguousarray(
            (-0.5 * sq[rows]).reshape(IT, 128).T
        ).astype(np.float32)                       # [128, IT]

        in_maps.append({
            "xl_hi": xl_hi, "xl_lo": xl_lo,
            "xr_hi": xr_hi, "xr_lo": xr_lo,
            "bias": bias,
        })
    return in_maps


def run(points: np.ndarray, **run_kwargs):
    """Run on HW; returns (K [4,4096,4096] f32, BassKernelResults)."""
    from concourse.bass_utils import run_bass_kernel_spmd

    nc = _get_nc()
    in_maps = _prep_inputs(points)
    res = run_bass_kernel_spmd(nc, in_maps, core_ids=list(range(N_CORES)),
                               **run_kwargs)
    out = np.empty((B, N, N), np.float32)
    for c in range(N_CORES):
        b, h = divmod(c, 2)
        out[b, h * HALF : (h + 1) * HALF, :] = res.results[c]["out"]
    return out, res


def kernel(points: np.ndarray) -> np.ndarray:
    out, _ = run(points)
    return out



# revision 3
# speedup vs baseline: 1.9532x; 1.9532x over previous
"""Gaussian kernel matrix K = exp(-|xi-xj|^2/2) on 8 TRN2 NeuronCores.

Input : points [4, 4096, 64] f32
Output: K      [4, 4096, 4096] f32

Sharding: core c handles batch c//2, row half c%2 (2048 rows x 4096 cols).

Math: -d2/2 = xi.xj - |xi|^2/2 - |xj|^2/2.
A single fp16 matmul pass computes  inner_aug[i,j] = xi.xj - |xj|^2/2
via two augmented contraction rows (lhsT rows 64,65 = ones; rhs row 64 =
fp16 hi of -|xj|^2/2, row 65 = fp16 lo), K=66.  fp16's 11 significant
bits give ~1.6e-3 L2 error - inside the 2e-2 gate - at 1/3 the PE time
of a 3-pass bf16 hi/lo scheme.
The -|xi|^2/2 term is the per-partition fp32 bias of the ScalarE
activation:  K = Exp(1.0*psum + bias_i), written as bf16 (host upcasts),
halving the output DMA bytes.
"""

import numpy as np

B, N, D = 4, 4096, 64
KA = D + 2          # contraction dim: 64 dims + hi/lo aug rows
HALF = N // 2       # rows per core
N_CORES = 8
IT = HALF // 128    # 16 i-tiles of 128 partitions
JG = N // 2048      # 2 j-groups of 2048 (4 PSUM banks)
JS = 2048 // 512    # 4 matmul sub-tiles per j-group

_cache = {}


NBUF_OUT = 3   # SBUF staging buffers for the output tiles
NGRP = IT * JG  # 32 pipeline groups of [128, 2048]


def _build_nc():
    import concourse.bass as bass
    import concourse.mybir as mybir

    f32 = mybir.dt.float32
    f16 = mybir.dt.float16
    bf16 = mybir.dt.bfloat16
    Exp = mybir.ActivationFunctionType.Exp

    nc = bass.Bass()
    xl_d = nc.dram_tensor("xl", [KA, HALF], f16, kind="ExternalInput")
    xr_d = nc.dram_tensor("xr", [KA, N], f16, kind="ExternalInput")
    bias_d = nc.dram_tensor("bias", [128, IT], f32, kind="ExternalInput")
    out_d = nc.dram_tensor("out", [HALF, N], bf16, kind="ExternalOutput")

    with (
        nc.sbuf_tensor([KA, HALF], f16) as xl,
        nc.sbuf_tensor([KA, N], f16) as xr,
        nc.sbuf_tensor([128, IT], f32) as bias,
        nc.sbuf_tensor([128, NBUF_OUT * 4096], bf16) as ot_buf,
        nc.psum_tensor([128, 2048], f32) as ps0,
        nc.psum_tensor([128, 2048], f32) as ps1,
        nc.semaphore("in_sem") as in_sem,
        nc.semaphore("mm_sem") as mm_sem,
        nc.semaphore("act_sem") as act_sem,
        nc.semaphore("out_sem_a") as out_sem_a,
        nc.semaphore("out_sem_b") as out_sem_b,
        nc.Block() as block,
    ):
        pss = [ps0, ps1]

        def ot(g):
            # out staging: one [128, 4096] buffer per it-block, slot it%NBUF
            it, jg = divmod(g, JG)
            s = (it % NBUF_OUT) * 4096 + jg * 2048
            return ot_buf[:, s : s + 2048]

        @block.sync
        def _(sync):
            sync.dma_start(out=xl[:], in_=xl_d[:, :]).then_inc(in_sem, 16)
            sync.dma_start(out=xr[:], in_=xr_d[:, :]).then_inc(in_sem, 16)
            sync.dma_start(out=bias[:], in_=bias_d[:, :]).then_inc(in_sem, 16)
            for it in range(IT):
                # jg=0 half of each it-block goes out on the SP HWDGE ring;
                # the jg=1 half is issued from the ACT engine's ring below.
                sync.wait_ge(act_sem, JG * it + 1)
                s = (it % NBUF_OUT) * 4096
                sync.dma_start(
                    out=out_d[it * 128 : (it + 1) * 128, 0:2048],
                    in_=ot_buf[:, s : s + 2048],
                ).then_inc(out_sem_a, 16)

        @block.tensor
        def _(tensor):
            tensor.wait_ge(in_sem, 48)
            for g in range(NGRP):
                it, jg = divmod(g, JG)
                if g >= 2:
                    # psum slot g%2 was last read by group g-2's activation
                    tensor.wait_ge(act_sem, g - 1)
                ps = pss[g % 2]
                lh = xl[:, it * 128 : (it + 1) * 128]
                last = None
                for js in range(JS):
                    c0 = jg * 2048 + js * 512
                    dst = ps[:, js * 512 : (js + 1) * 512]
                    last = tensor.matmul(dst, lh, xr[:, c0 : c0 + 512],
                                         start=True, stop=True)
                # PE completes in order; one inc on the last matmul
                last.then_inc(mm_sem, 1)

        @block.scalar
        def _(scalar):
            scalar.wait_ge(in_sem, 48)
            for g in range(NGRP):
                it, jg = divmod(g, JG)
                ps = pss[g % 2]
                scalar.wait_ge(mm_sem, g + 1)
                if jg == 0 and it >= NBUF_OUT:
                    # out slot it%NBUF_OUT last read by DMAs of it-NBUF_OUT
                    scalar.wait_ge(out_sem_a, 16 * (it - NBUF_OUT + 1))
                    scalar.wait_ge(out_sem_b, 16 * (it - NBUF_OUT + 1))
                o = ot(g)
                scalar.activation(
                    o[:, :], ps[:, :], Exp,
                    bias=bias[:, it : it + 1], scale=1.0,
                ).then_inc(act_sem, 1)
                if jg == 1:
                    # own-engine wait: the activation's write must retire
                    # before the ACT-ring DMA reads the staging buffer
                    scalar.wait_ge(act_sem, g + 1)
                    s = (it % NBUF_OUT) * 4096
                    scalar.dma_start(
                        out=out_d[it * 128 : (it + 1) * 128, 2048:4096],
                        in_=ot_buf[:, s + 2048 : s + 4096],
                    ).then_inc(out_sem_b, 16)
    return nc


def _get_nc():
    if "nc" not in _cache:
        _cache["nc"] = _build_nc()
    return _cache["nc"]


def _prep_inputs(points: np.ndarray):
    """Host-side shard/layout prep: per-core transposed + augmented operands."""
    points = np.asarray(points, dtype=np.float32)
    in_maps = []
    xr_cache = {}
    for c in range(N_CORES):
        b, h = divmod(c, 2)
        if b not in xr_cache:
            x = points[b]                              # [N, D]
            sq = np.sum(x * x, axis=1)                 # [N]
            xt = np.ascontiguousarray(x.T)             # [D, N]
            aug_hi = (-0.5 * sq).astype(np.float16)
            aug_lo = ((-0.5 * sq) - aug_hi.astype(np.float32)).astype(np.float16)
            xr = np.empty((KA, N), np.float16)
            xr[:D] = xt
            xr[D] = aug_hi
            xr[D + 1] = aug_lo
            xr_cache[b] = (xr, sq)
        xr, sq = xr_cache[b]

        rows = slice(h * HALF, (h + 1) * HALF)
        xl = np.empty((KA, HALF), np.float16)
        xl[:D] = xr[:D, rows]
        xl[D] = 1.0
        xl[D + 1] = 1.0

        bias = np.ascontiguousarray(
            (-0.5 * sq[rows]).reshape(IT, 128).T
        ).astype(np.float32)                           # [128, IT]

        in_maps.append({"xl": xl, "xr": xr, "bias": bias})
    return in_maps


def run(points: np.ndarray, **run_kwargs):
    """Run on HW; returns (K [4,4096,4096] f32, BassKernelResults)."""
    from concourse.bass_utils import run_bass_kernel_spmd

    nc = _get_nc()
    in_maps = _prep_inputs(points)
    res = run_bass_kernel_spmd(nc, in_maps, core_ids=list(range(N_CORES)),
                               **run_kwargs)
    out = np.empty((B, N, N), np.float32)
    for c in range(N_CORES):
        b, h = divmod(c, 2)
        out[b, h * HALF : (h + 1) * HALF, :] = res.results[c]["out"].astype(
            np.float32
        )
    return out, res


def kernel(points: np.ndarray) -> np.ndarray:
    out, _ = run(points)
    return out


# revision 4
# speedup vs baseline: 2.9102x; 1.4900x over previous
"""Gaussian kernel matrix K = exp(-|xi-xj|^2/2) on 8 TRN2 NeuronCores,
exploiting symmetry: each core computes only lower-triangle block rows.

Input : points [4, 4096, 64] f32
Output: K      [4, 4096, 4096] f32

K[b] is symmetric: only the lower block triangle (block row r of 128
rows spans columns 0..(r+1)*128) is computed on-device; the host
mirrors the strict upper triangle.

Sharding (SPMD-uniform): core c = (batch c//2, h = c%2) takes pairs
(r, 31-r) for r = h, h+2, ..., h+14.  Widths are padded up to 512
multiples, which makes the padded pair shape IDENTICAL for all cores:
pair slot q has lo width L_q=(q//2+1)*512, hi width H_q=4608-L_q, so
every pair strip is [128, 4608] and all 8 cores run the same program
(~4.61M padded outputs/core vs 8.39M for the full-matrix split).

Math: -d2/2 = xi.xj - |xi|^2/2 - |xj|^2/2, one fp16 matmul pass with
two augmented contraction rows (ones | fp16 hi/lo of -|xj|^2/2), K=66.
-|xi|^2/2 enters as the fp32 per-partition ScalarE bias:
K = Exp(psum + bias_i), emitted bf16 (host upcasts).
"""

import numpy as np

B, N, D = 4, 4096, 64
KA = D + 2          # contraction dim: 64 dims + hi/lo aug rows
N_CORES = 8
NBLK = N // 128     # 32 block rows per batch
NPAIR = 8           # pairs per core
PW = 4608           # padded columns per pair strip (L_q + H_q)
GPP = 3             # chunk groups per pair (lo: 1, hi: 2048 + rest)
NGRP = NPAIR * GPP  # 24 pipeline groups per core

NBUF_OUT = 3        # staging strips of [128, PW]

_cache = {}


def _lo_w(q):
    return (q // 2 + 1) * 512   # padded lo width: 512,512,1024,...,2048


def _pairs(h):
    return [(h + 2 * q, 31 - (h + 2 * q)) for q in range(NPAIR)]


def _groups():
    """Static schedule, identical for every core: one entry per
    (pair, block, col-chunk): bi = lhs block slot, c0 = rhs col start,
    cw = chunk width, off = staging col offset within the pair strip."""
    gs = []
    for q in range(NPAIR):
        L = _lo_w(q)
        H = PW - L
        gs.append(dict(q=q, bi=2 * q, c0=0, cw=L, off=0))
        gs.append(dict(q=q, bi=2 * q + 1, c0=0, cw=2048, off=L))
        gs.append(dict(q=q, bi=2 * q + 1, c0=2048, cw=H - 2048,
                       off=L + 2048))
    assert len(gs) == NGRP
    return gs


def _build_nc():
    import concourse.bass as bass
    import concourse.mybir as mybir

    f32 = mybir.dt.float32
    f16 = mybir.dt.float16
    bf16 = mybir.dt.bfloat16
    Exp = mybir.ActivationFunctionType.Exp

    groups = _groups()

    nc = bass.Bass()
    xl_d = nc.dram_tensor("xl", [KA, 16 * 128], f16, kind="ExternalInput")
    xr_d = nc.dram_tensor("xr", [KA, N], f16, kind="ExternalInput")
    bias_d = nc.dram_tensor("bias", [128, 16], f32, kind="ExternalInput")
    out_d = nc.dram_tensor("out", [NPAIR * 128, PW], bf16,
                           kind="ExternalOutput")

    with (
        nc.sbuf_tensor([KA, 16 * 128], f16) as xl,
        nc.sbuf_tensor([KA, N], f16) as xr,
        nc.sbuf_tensor([128, 16], f32) as bias,
        nc.sbuf_tensor([128, NBUF_OUT * PW], bf16) as ot_buf,
        nc.psum_tensor([128, 2048], f32) as ps0,
        nc.psum_tensor([128, 2048], f32) as ps1,
        nc.semaphore("in_sem") as in_sem,
        nc.semaphore("mm_sem") as mm_sem,
        nc.semaphore("act_sem") as act_sem,
        nc.semaphore("out_sem_a") as out_sem_a,
        nc.semaphore("out_sem_b") as out_sem_b,
        nc.Block() as block,
    ):
        pss = [ps0, ps1]
        HPW = PW // 2  # per-ring DMA split of a pair strip

        @block.sync
        def _(sync):
            sync.dma_start(out=xl[:], in_=xl_d[:, :]).then_inc(in_sem, 16)
            sync.dma_start(out=xr[:], in_=xr_d[:, :]).then_inc(in_sem, 16)
            sync.dma_start(out=bias[:], in_=bias_d[:, :]).then_inc(in_sem, 16)
            for p in range(NPAIR):
                # strip halves: cols 0:HPW on the SP ring, rest on ACT ring
                sync.wait_ge(act_sem, GPP * (p + 1))
                s = (p % NBUF_OUT) * PW
                sync.dma_start(
                    out=out_d[p * 128 : (p + 1) * 128, 0:HPW],
                    in_=ot_buf[:, s : s + HPW],
                ).then_inc(out_sem_a, 16)

        @block.tensor
        def _(tensor):
            tensor.wait_ge(in_sem, 48)
            for g, gr in enumerate(groups):
                if g >= 2:
                    # psum slot g%2 was last read by group g-2's activation
                    tensor.wait_ge(act_sem, g - 1)
                ps = pss[g % 2]
                lh = xl[:, gr["bi"] * 128 : (gr["bi"] + 1) * 128]
                last = None
                for c in range(0, gr["cw"], 512):
                    last = tensor.matmul(
                        ps[:, c : c + 512],
                        lh,
                        xr[:, gr["c0"] + c : gr["c0"] + c + 512],
                        start=True, stop=True,
                    )
                last.then_inc(mm_sem, 1)

        @block.scalar
        def _(scalar):
            scalar.wait_ge(in_sem, 48)
            for g, gr in enumerate(groups):
                ps = pss[g % 2]
                scalar.wait_ge(mm_sem, g + 1)
                p = gr["q"]
                if g % GPP == 0 and p >= NBUF_OUT:
                    # staging slot p%NBUF_OUT last read by DMAs of p-NBUF_OUT
                    scalar.wait_ge(out_sem_a, 16 * (p - NBUF_OUT + 1))
                    scalar.wait_ge(out_sem_b, 16 * (p - NBUF_OUT + 1))
                s = (p % NBUF_OUT) * PW + gr["off"]
                scalar.activation(
                    ot_buf[:, s : s + gr["cw"]], ps[:, 0 : gr["cw"]], Exp,
                    bias=bias[:, gr["bi"] : gr["bi"] + 1], scale=1.0,
                ).then_inc(act_sem, 1)
                if g % GPP == GPP - 1:
                    # own-engine wait: all 3 activations of pair p must
                    # retire before the ACT-ring DMA reads staging
                    scalar.wait_ge(act_sem, g + 1)
                    s0 = (p % NBUF_OUT) * PW
                    scalar.dma_start(
                        out=out_d[p * 128 : (p + 1) * 128, HPW:PW],
                        in_=ot_buf[:, s0 + HPW : s0 + PW],
                    ).then_inc(out_sem_b, 16)
    return nc


def _get_nc():
    if "nc" not in _cache:
        _cache["nc"] = _build_nc()
    return _cache["nc"]


def _prep_inputs(points: np.ndarray):
    """Host-side shard/layout prep: per-core transposed + augmented operands."""
    points = np.asarray(points, dtype=np.float32)
    per_batch = {}
    for b in range(B):
        x = points[b]                              # [N, D]
        sq = np.sum(x * x, axis=1)                 # [N]
        xt = np.ascontiguousarray(x.T)             # [D, N]
        aug_hi = (-0.5 * sq).astype(np.float16)
        aug_lo = ((-0.5 * sq) - aug_hi.astype(np.float32)).astype(np.float16)
        xr = np.empty((KA, N), np.float16)
        xr[:D] = xt
        xr[D] = aug_hi
        xr[D + 1] = aug_lo
        per_batch[b] = (xr, sq)

    in_maps = []
    for c in range(N_CORES):
        b, h = divmod(c, 2)
        xr, sq = per_batch[b]
        xl = np.empty((KA, 16 * 128), np.float16)
        bias = np.empty((128, 16), np.float32)
        for q, (r_lo, r_hi) in enumerate(_pairs(h)):
            for slot, r in ((2 * q, r_lo), (2 * q + 1, r_hi)):
                rows = slice(r * 128, (r + 1) * 128)
                xl[:D, slot * 128 : (slot + 1) * 128] = xr[:D, rows]
                bias[:, slot] = -0.5 * sq[rows]
        xl[D] = 1.0
        xl[D + 1] = 1.0
        in_maps.append({"xl": xl, "xr": xr, "bias": bias})
    return in_maps


def _assemble(results):
    """Unpack per-core strips, mirror the strict upper block triangle."""
    out = np.empty((B, N, N), np.float32)
    for c in range(N_CORES):
        b, h = divmod(c, 2)
        buf = results[c]["out"].astype(np.float32)   # [1024, PW]
        for q, (r_lo, r_hi) in enumerate(_pairs(h)):
            L = _lo_w(q)
            w_lo = (r_lo + 1) * 128
            w_hi = (r_hi + 1) * 128
            rows = buf[q * 128 : (q + 1) * 128]
            out[b, r_lo * 128 : (r_lo + 1) * 128, 0:w_lo] = rows[:, 0:w_lo]
            out[b, r_hi * 128 : (r_hi + 1) * 128, 0:w_hi] = \
                rows[:, L : L + w_hi]
    iu, ju = np.triu_indices(NBLK, 1)
    for b in range(B):
        v = out[b].reshape(NBLK, 128, NBLK, 128)
        v[iu, :, ju, :] = v[ju, :, iu, :].transpose(0, 2, 1)
    return out


def run(points: np.ndarray, **run_kwargs):
    """Run on HW; returns (K [4,4096,4096] f32, BassKernelResults)."""
    from concourse.bass_utils import run_bass_kernel_spmd

    nc = _get_nc()
    in_maps = _prep_inputs(points)
    res = run_bass_kernel_spmd(nc, in_maps, core_ids=list(range(N_CORES)),
                               **run_kwargs)
    return _assemble(res.results), res


def kernel(points: np.ndarray) -> np.ndarray:
    out, _ = run(points)
    return out
